# revision 4
# baseline (speedup 1.0000x reference)
"""Trainium2 Bass kernel for nn_MedianPool2d (K=3, stride=1, same-pad along W).

The reference op is a width-wise median-of-3 with replicate padding:
    out[..., w] = median(x[..., w-1], x[..., w], x[..., w+1])   (clamped at edges)
Replicate padding makes the edge columns pass-throughs: median(x0, x0, x1) == x0.

Strategy:
  - Shard batch across 8 NeuronCores (32 batches -> 4 per core), fully data
    parallel, no collectives.
  - Per core the shard is a flat [8192, 1024] row matrix (rows = b*H + h).
    Tiles of 128 partitions x R rows are DMAed to SBUF; the median network
    min/max ops run on the DVE with 3-D access patterns ([p, r, w]) so one
    instruction covers R rows while never crossing a row boundary.
  - median3(a,b,c) = max(min(a,b), min(max(a,b), c)) -> 4 tensor_tensor ops.
    The final op writes the interior columns back into the input tile in
    place; the untouched edge columns already hold x (the correct output),
    so no edge fixup instructions are needed.
  - Loads are issued on the SP (sync) HWDGE ring, stores on the ACT (scalar)
    HWDGE ring so the two directions never queue behind each other.
"""

import numpy as np

import concourse.bacc as bacc
import concourse.bass as bass
import concourse.mybir as mybir
import concourse.tile as tile
from concourse.alu_op_type import AluOpType
from concourse.bass_utils import run_bass_kernel_spmd

N_CORES = 8
B, C, H, W = 32, 1, 2048, 1024
P = 128                      # SBUF partitions
ROWS = (B // N_CORES) * C * H  # 8192 rows per core
R = 4                        # W-rows per partition per tile
TILE_ROWS = P * R            # 512
N_TILES = ROWS // TILE_ROWS  # 16
FP32 = mybir.dt.float32


def build_program(repeats: int = 1) -> bass.Bass:
    """Build the per-core Bass program (identical on all cores).

    repeats > 1 re-runs the whole kernel body back to back on the same DRAM
    buffers; used for wall-clock-delta timing since NTFF profiling is not
    available under this axon client.
    """
    nc = bacc.Bacc("TRN2", target_bir_lowering=False, debug=False)
    x_d = nc.dram_tensor("x", [ROWS, W], FP32, kind="ExternalInput").ap()
    y_d = nc.dram_tensor("y", [ROWS, W], FP32, kind="ExternalOutput").ap()

    with tile.TileContext(nc) as tc:
        with (
            tc.tile_pool(name="xt", bufs=3) as xpool,
            tc.tile_pool(name="ot", bufs=3) as opool,
            tc.tile_pool(name="tmp", bufs=2) as tpool,
        ):
            for _rep in range(repeats):
                for t in range(N_TILES):
                    rows = slice(t * TILE_ROWS, (t + 1) * TILE_ROWS)
                    src = x_d[rows, :].rearrange("(p r) w -> p (r w)", p=P)
                    dst = y_d[rows, :].rearrange("(p r) w -> p (r w)", p=P)

                    xt = xpool.tile([P, R * W], FP32, tag="xt")
                    nc.sync.dma_start(out=xt[:], in_=src)

                    x3 = xt.rearrange("p (r w) -> p r w", w=W)
                    a = x3[:, :, 0 : W - 2]
                    b = x3[:, :, 1 : W - 1]
                    c = x3[:, :, 2:W]

                    # Output tile is written entirely by the DVE so the store
                    # DMA needs exactly one sem wait (walrus HWDGE DMA codegen
                    # rejects DMAs with >1 sync wait).
                    ot = opool.tile([P, R * W], FP32, tag="ot")
                    o3 = ot.rearrange("p (r w) -> p r w", w=W)
                    o3i = o3[:, :, 1 : W - 1]

                    tmax = tpool.tile([P, R * (W - 2)], FP32, tag="tmax")
                    tmax3 = tmax.rearrange("p (r w) -> p r w", w=W - 2)

                    nc.vector.tensor_tensor(out=o3i, in0=a, in1=b, op=AluOpType.min)
                    nc.vector.tensor_tensor(out=tmax3, in0=a, in1=b, op=AluOpType.max)
                    nc.vector.tensor_tensor(
                        out=tmax3, in0=tmax3, in1=c, op=AluOpType.min
                    )
                    nc.vector.tensor_tensor(
                        out=o3i, in0=o3i, in1=tmax3, op=AluOpType.max
                    )
                    # replicate-pad edges: out[..., 0] = x[..., 0],
                    # out[..., W-1] = x[..., W-1]; one strided copy covers both
                    nc.vector.tensor_copy(
                        out=o3[:, :, 0 : W : W - 1], in_=x3[:, :, 0 : W : W - 1]
                    )

                    nc.sync.dma_start(out=dst, in_=ot[:])
    nc.compile()
    return nc


def run_sharded(x: np.ndarray, repeats: int = 1) -> np.ndarray:
    """Shard [B,C,H,W] input across 8 cores, run, gather to full output."""
    x = np.ascontiguousarray(np.asarray(x), dtype=np.float32)
    assert x.shape == (B, C, H, W), x.shape
    shards = x.reshape(N_CORES, ROWS, W)
    nc = build_program(repeats=repeats)
    in_maps = [{"x": shards[i]} for i in range(N_CORES)]
    res = run_bass_kernel_spmd(nc, in_maps, core_ids=list(range(N_CORES))).results
    out = np.stack([res[i]["y"] for i in range(N_CORES)], axis=0)
    return out.reshape(B, C, H, W)


def kernel(x: np.ndarray) -> np.ndarray:
    return run_sharded(x, repeats=1)


# revision 11
# speedup vs baseline: 1.0076x; 1.0076x over previous
"""Trainium2 Bass kernel for nn_MedianPool2d (K=3, stride=1, same-pad along W).

The reference op is a width-wise median-of-3 with replicate padding:
    out[..., w] = median(x[..., w-1], x[..., w], x[..., w+1])   (clamped at edges)
Replicate padding makes the edge columns pass-throughs: median(x0, x0, x1) == x0.

Strategy:
  - Shard batch across 8 NeuronCores (32 batches -> 4 per core), fully data
    parallel, no collectives.
  - Per core the shard is a flat [8192, 1024] row matrix (rows = b*H + h).
    Tiles of 128 partitions x R rows are DMAed to SBUF; the median network
    min/max ops run on the DVE with 3-D access patterns ([p, r, w]) so one
    instruction covers R rows while never crossing a row boundary.
  - median3(a,b,c) = max(min(a,b), min(max(a,b), c)) -> 4 tensor_tensor ops.
    The final op writes the interior columns back into the input tile in
    place; the untouched edge columns already hold x (the correct output),
    so no edge fixup instructions are needed.
  - Loads are issued on the SP (sync) HWDGE ring, stores on the ACT (scalar)
    HWDGE ring so the two directions never queue behind each other.
"""

import numpy as np

import concourse.bacc as bacc
import concourse.bass as bass
import concourse.mybir as mybir
import concourse.tile as tile
from concourse.alu_op_type import AluOpType
from concourse.bass_utils import run_bass_kernel_spmd

N_CORES = 8
B, C, H, W = 32, 1, 2048, 1024
P = 128                      # SBUF partitions
ROWS = (B // N_CORES) * C * H  # 8192 rows per core
R = 4                        # W-rows per partition per tile
TILE_ROWS = P * R            # 512
N_TILES = ROWS // TILE_ROWS  # 16
FP32 = mybir.dt.float32


def build_program(
    repeats: int = 1,
    r: int = R,
    flat: bool = False,
    dual_ring: bool = False,
    do_compute: bool = True,
    do_dma: bool = True,
    do_store: bool = True,
    edge_copy: bool = True,
    edge_engine: str = "vector",
    bufs: tuple[int, int, int] = (3, 3, 2),
) -> bass.Bass:
    """Build the per-core Bass program (identical on all cores).

    repeats > 1 re-runs the whole kernel body back to back on the same DRAM
    buffers; used for wall-clock-delta timing since NTFF profiling is not
    available under this axon client. The other knobs exist for bottleneck
    experiments (DMA-only / DVE-only / AP-shape variants).
    """
    tile_rows = P * r
    n_tiles = ROWS // tile_rows
    nc = bacc.Bacc("TRN2", target_bir_lowering=False, debug=False)
    x_d = nc.dram_tensor("x", [ROWS, W], FP32, kind="ExternalInput").ap()
    y_d = nc.dram_tensor("y", [ROWS, W], FP32, kind="ExternalOutput").ap()

    with tile.TileContext(nc) as tc:
        with (
            tc.tile_pool(name="xt", bufs=bufs[0]) as xpool,
            tc.tile_pool(name="ot", bufs=bufs[1]) as opool,
            tc.tile_pool(name="tmp", bufs=bufs[2]) as tpool,
        ):
            for _rep in range(repeats):
                for t in range(n_tiles):
                    rows = slice(t * tile_rows, (t + 1) * tile_rows)
                    src = x_d[rows, :].rearrange("(p r) w -> p (r w)", p=P)
                    dst = y_d[rows, :].rearrange("(p r) w -> p (r w)", p=P)

                    xt = xpool.tile([P, r * W], FP32, tag="xt")
                    if do_dma:
                        nc.sync.dma_start(out=xt[:], in_=src)

                    x3 = xt.rearrange("p (r w) -> p r w", w=W)

                    # Output tile is written entirely by the DVE so the store
                    # DMA needs exactly one sem wait (walrus HWDGE DMA codegen
                    # rejects DMAs with >1 sync wait).
                    if do_compute:
                        ot = opool.tile([P, r * W], FP32, tag="ot", name="ot")
                    else:
                        ot = xt
                    o3 = ot.rearrange("p (r w) -> p r w", w=W)

                    if do_compute:
                        tmax = tpool.tile(
                            [P, r * W if flat else r * (W - 2)], FP32, tag="tmax"
                        )
                        if flat:
                            # 2-D APs across the whole tile: row-seam columns
                            # compute garbage, but every wrong column is a
                            # replicate-pad pass-through that the strided edge
                            # copy below overwrites.
                            n = r * W - 2
                            a = xt[:, 0:n]
                            b = xt[:, 1 : n + 1]
                            c = xt[:, 2 : n + 2]
                            oi = ot[:, 1 : n + 1]
                            tm = tmax[:, 0:n]
                        else:
                            a = x3[:, :, 0 : W - 2]
                            b = x3[:, :, 1 : W - 1]
                            c = x3[:, :, 2:W]
                            oi = o3[:, :, 1 : W - 1]
                            tm = tmax.rearrange("p (r w) -> p r w", w=W - 2)

                        nc.vector.tensor_tensor(out=oi, in0=a, in1=b, op=AluOpType.min)
                        nc.vector.tensor_tensor(out=tm, in0=a, in1=b, op=AluOpType.max)
                        nc.vector.tensor_tensor(out=tm, in0=tm, in1=c, op=AluOpType.min)
                        nc.vector.tensor_tensor(out=oi, in0=oi, in1=tm, op=AluOpType.max)
                        # replicate-pad edges (and, for flat=True, row seams):
                        # out[..., 0] = x[..., 0], out[..., W-1] = x[..., W-1];
                        # one strided copy covers both columns of all r rows
                        if edge_copy:
                            if edge_engine == "scalar":
                                nc.scalar.copy(
                                    out=o3[:, :, 0 : W : W - 1],
                                    in_=x3[:, :, 0 : W : W - 1],
                                )
                            else:
                                nc.vector.tensor_copy(
                                    out=o3[:, :, 0 : W : W - 1],
                                    in_=x3[:, :, 0 : W : W - 1],
                                )

                    if do_dma and do_store:
                        eng = nc.scalar if (dual_ring and t % 2) else nc.sync
                        eng.dma_start(out=dst, in_=ot[:])
    nc.compile()
    return nc


_NC_CACHE: dict[int, bass.Bass] = {}


def run_sharded(x: np.ndarray, repeats: int = 1) -> np.ndarray:
    """Shard [B,C,H,W] input across 8 cores, run, gather to full output."""
    x = np.ascontiguousarray(np.asarray(x), dtype=np.float32)
    assert x.shape == (B, C, H, W), x.shape
    shards = x.reshape(N_CORES, ROWS, W)
    nc = _NC_CACHE.get(repeats)
    if nc is None:
        nc = _NC_CACHE[repeats] = build_program(repeats=repeats)
    in_maps = [{"x": shards[i]} for i in range(N_CORES)]
    res = run_bass_kernel_spmd(nc, in_maps, core_ids=list(range(N_CORES))).results
    out = np.stack([res[i]["y"] for i in range(N_CORES)], axis=0)
    return out.reshape(B, C, H, W)


def kernel(x: np.ndarray) -> np.ndarray:
    return run_sharded(x, repeats=1)
